# revision 15
# baseline (speedup 1.0000x reference)
"""TRN2 Bass kernel for nn_Denoise: 2x100-iteration FISTA row denoise.

Scheme (per core, data-parallel over batch):
  Layout: transposed per image -> tiles [128 elem-partitions, 8192] where
  column = chunk*2048 + img*512 + row  (4 chunks of 128 row-elements).
  Iteration: v = k1*z - k2*(A @ f32r(z)) + s*y   (A = D^T D, integer entries,
  exact in float32r; computed on PE accumulating over ACT-preloaded alpha*y),
  m = min(v, y) ;  x = relu(m) ;  z' = (1+b)*relu(m) - b*relu(m_prev).
  The fp32 k1*z path keeps full precision; quantization error A*(z - f32r(z))
  is spectrally annihilated where the QP resolvent would amplify it.
"""
import numpy as np

LAM = 10.0
N_ITERS = 100
N = 512
NIMG_PER_CORE = 4
NCORES = 8
FD = NIMG_PER_CORE * N          # 2048 columns per chunk
NCHUNK = 4                      # 512 = 4 * 128 row-elements
TOT = NCHUNK * FD               # 8192
G = 4                           # groups (psum pipelining)
GFD = FD // G                   # 512 columns per chunk per group

_cache = {}


def _f32r(v):
    b = np.ascontiguousarray(v, np.float32).view(np.uint32)
    r = b + 0x7FF + ((b >> 12) & 1)
    r &= np.uint32(0xFFFFF000)
    return r.view(np.float32)


def _host_consts():
    step = np.float32(1.0 / (2.0 * (1.0 + 16.0 * LAM)))
    k1 = float(1.0 - 2.0 * float(step))
    k2 = float(2.0 * LAM * float(step))
    s = float(2.0 * float(step))
    alpha = -s / k2  # = -1/LAM
    # beta sequence in fp32 exactly like the reference
    t = np.float32(1.0)
    b0 = np.zeros(N_ITERS, np.float32)
    b1 = np.zeros(N_ITERS, np.float32)
    for k in range(N_ITERS):
        tn = np.float32(0.5) * (np.float32(1.0) + np.sqrt(np.float32(1.0) + np.float32(4.0) * t * t))
        beta = (t - np.float32(1.0)) / tn
        b0[k] = beta
        b1[k] = np.float32(1.0) + beta
        t = tn
    # A = D^T D (integer entries)
    D = np.zeros((N - 2, N), np.float64)
    idx = np.arange(N - 2)
    D[idx, idx] = 1.0
    D[idx, idx + 1] = -2.0
    D[idx, idx + 2] = 1.0
    A = D.T @ D
    # distinct lhsT blocks: lhsT[k, m] = A[jbase + k, ibase + m]
    A00 = A[0:128, 0:128]
    AII = A[128:256, 128:256]
    A33 = A[384:512, 384:512]
    BU = A[128:256, 0:128]      # cj = c+1 feeding c
    BL = A[0:128, 128:256]      # cj = c-1 feeding c
    wts = np.concatenate([A00, AII, A33, BU, BL, np.eye(128)], axis=1).astype(np.float32)  # [128, 768]
    betas = np.concatenate([b1, b0])[None, :].repeat(128, axis=0).astype(np.float32)  # [128, 200]
    return step, k1, k2, s, alpha, wts, betas


def _register_ops():
    from concourse import dve_ops
    from concourse.dve_spec import Spec, Src0, Src1, C0, C1, lower
    from concourse.dve_spec import _has_src1 as has_src1
    from concourse.dve_spec import relu as drelu
    from concourse.dve_uop import DveOpSpec

    def register_op(name, spec):
        for o in dve_ops.OPS:
            if o.name == name:
                return o
        op = dve_ops.DveOp(name, spec, False, uops_sha={})
        dve_ops.OPS.append(op)
        row = dve_ops._CUSTOM_DVE_ROW_BASE + len(dve_ops.OPS) - 1
        assert row < 0x20
        dve_ops._SUB_OPCODE_FOR_NAME[name] = row
        dve_ops.CUSTOM_DVE_SPECS[name] = spec
        sha = {}
        for ver in ("v3", "v4"):
            sp = DveOpSpec(name=name, opcode=row, uops=lower(spec, ver=ver),
                           rd1_en=has_src1(spec))
            sha[ver] = sp.sha(ver)
        op2 = dve_ops.DveOp(name, spec, False, uops_sha=sha)
        dve_ops.OPS[-1] = op2
        return op2

    fista_v = register_op(
        "FISTA_V",
        Spec(body=(Src0 - Src1 * C0) * C1,
             reference=lambda in0, in1, s0, s1, imm2: (in0 - in1 * s0) * s1))
    from concourse.dve_spec import minn
    fista_m = register_op(
        "FISTA_M",
        Spec(body=minn(Src0 + Src1 * C0, Src1),
             reference=lambda in0, in1, s0, s1, imm2: np.minimum(in0 + in1 * s0, in1)))
    fista_z = register_op(
        "FISTA_Z",
        Spec(body=drelu(Src0) * C0 - drelu(Src1) * C1,
             reference=lambda in0, in1, s0, s1, imm2:
             np.maximum(in0, 0) * s0 - np.maximum(in1, 0) * s1))
    # v3 ops: x = relu(min(v + s*y, y))  (M with relu fused);  z' = b1*x_new - b0*x_old
    fista_x = register_op(
        "FISTA_XR",
        Spec(body=drelu(minn(Src0 + Src1 * C0, Src1)),
             reference=lambda in0, in1, s0, s1, imm2:
             np.maximum(np.minimum(in0 + in1 * s0, in1), 0)))
    fista_zp = register_op(
        "FISTA_ZP",
        Spec(body=Src0 * C0 - Src1 * C1,
             reference=lambda in0, in1, s0, s1, imm2: in0 * s0 - in1 * s1))
    return fista_v, fista_m, fista_z, fista_x, fista_zp


N_ITERS_RUN = [N_ITERS]
MM_MODE = ["fold"]  # "fold" = v2 (k1*z in PE weights, 2 DVE ops/chunk); "full" = legacy
DUMP_PS = [False]
RDT = ["f32r"]   # rounding dtype for z/weights: f32r or bf16
NHALF = [1]
SKIP = [set()]
LOOP_MODE = ["dynamic"]
PASSES = [2]
BODY = [2]       # FISTA iterations per dynamic-loop body (must divide N_ITERS_RUN)

def _build(trace=False):
    key = (N_ITERS_RUN[0], LOOP_MODE[0], PASSES[0], MM_MODE[0], DUMP_PS[0], tuple(sorted(SKIP[0])), RDT[0], NHALF[0], BODY[0])
    if key in _cache:
        return _cache[key]
    import concourse.bacc as bacc
    import concourse.tile as tile
    from concourse import mybir
    import concourse.bass as bass

    FISTA_V, FISTA_M, FISTA_Z, FISTA_X, FISTA_ZP = _register_ops()
    step, k1, k2, s, alpha, wts_np, betas_np = _host_consts()
    f32 = mybir.dt.float32
    f32r = mybir.dt.float32r if RDT[0] == "f32r" else mybir.dt.bfloat16

    if MM_MODE[0] == "fold":
        return _build_fold(nc_args=(bacc, tile, mybir, bass),
                           ops=(FISTA_V, FISTA_X, FISTA_ZP),
                           consts=(k1, k2, s), key=key)

    nc = bacc.Bacc("TRN2", target_bir_lowering=False, debug=False)
    DATA = nc.dram_tensor("data", [NIMG_PER_CORE, N, N], f32, kind="ExternalInput")
    WTS = nc.dram_tensor("wts", [128, 6 * 128], f32, kind="ExternalInput")
    BET = nc.dram_tensor("betas", [128, 2 * N_ITERS], f32, kind="ExternalInput")
    OUT = nc.dram_tensor("out", [NIMG_PER_CORE, N, N], f32, kind="ExternalOutput")

    with tile.TileContext(nc) as tc:
        with (
            tc.tile_pool(name="state", bufs=1) as pool,
            tc.tile_pool(name="psum", bufs=2, space="PSUM") as psp,
            tc.tile_pool(name="scratch", bufs=1) as pool2,
        ):
            y_t = pool.tile([128, TOT], f32, tag="y")
            z_t = pool.tile([128, TOT], f32, tag="z")
            zr_a = pool.tile([128, TOT], f32r, tag="zra")
            zr_b = pool.tile([128, TOT], f32r, tag="zrb")
            m_a = pool.tile([128, TOT], f32, tag="ma")
            m_b = pool.tile([128, TOT], f32, tag="mb")
            w_t = pool.tile([128, 6 * 128], f32, tag="w")
            wr_t = pool.tile([128, 5 * 128], f32r, tag="wr")
            stg_pool = pool
            bet_t = pool.tile([128, 2 * N_ITERS], f32, tag="bet")

            # ---- load weights/betas
            nc.sync.dma_start(w_t[:], WTS[:])
            nc.sync.dma_start(bet_t[:], BET[:])
            nc.vector.tensor_copy(wr_t[:], w_t[:, :5 * 128])
            ident = w_t[:, 5 * 128:6 * 128]
            WBLK = {  # (cj - c) -> per-c lhsT slice index into wr_t
                ("d", 0): 0, ("d", 1): 1, ("d", 2): 1, ("d", 3): 2,
            }

            def wslice(idx):
                return wr_t[:, idx * 128:(idx + 1) * 128]

            # ---- load input + transpose on PE:
            # y[p, c*FD + i*N + (128t+r')] = data[i, 128t+r', 128c+p]
            for i in range(NIMG_PER_CORE):
                for t_ in range(4):
                    sbase = ((i * 4 + t_) % 4) * N
                    nc.sync.dma_start(m_b[:, sbase:sbase + N],
                                      DATA[i, 128 * t_:128 * (t_ + 1), :])
                    psT = psp.tile([128, NCHUNK * GFD], f32, tag="ps")
                    for c in range(NCHUNK):
                        nc.tensor.transpose(psT[:, 128 * c:128 * (c + 1)],
                                            m_b[:, sbase + 128 * c:sbase + 128 * (c + 1)], ident)
                    dstv = y_t[:].rearrange("p (c n) -> p c n", c=NCHUNK)[
                        :, :, i * N + 128 * t_: i * N + 128 * (t_ + 1)]
                    nc.vector.tensor_copy(
                        dstv, psT[:, :N].rearrange("p (c n) -> p c n", c=NCHUNK))

            def init_state():
                nc.vector.tensor_copy(z_t[:], y_t[:])
                nc.gpsimd.tensor_copy(zr_a[:], y_t[:])
                nc.scalar.copy(m_a[:], y_t[:])

            def g3(t, g):
                """[128, 4, GFD] view of group g of a [128, TOT] tile."""
                return t[:].rearrange("p (c n) -> p c n", c=NCHUNK)[:, :, g * GFD:(g + 1) * GFD]

            HW = FD // NHALF[0]   # interleaved independent column groups

            def iteration(m_in, m_out, zr_in, zr_out, h, i_b1, i_b0):
                hb = h * HW
                for c in range(NCHUNK):
                    cs = slice(c * FD + hb, c * FD + hb + HW)
                    ps = psp.tile([128, HW], mybir.dt.float32, tag="ps")
                    mlist = [(WBLK[("d", c)], c)]
                    if "offdiag" not in SKIP[0]:
                        if c + 1 < NCHUNK:
                            mlist.append((3, c + 1))
                        if c - 1 >= 0:
                            mlist.append((4, c - 1))
                    if "mm" in SKIP[0]:
                        mlist = []
                        nc.scalar.activation(ps[:], y_t[:, cs], mybir.ActivationFunctionType.Copy)
                    for p in range(HW // 512):
                        for bi, (widx, cj) in enumerate(mlist):
                            nc.tensor.matmul(
                                ps[:, p * 512:(p + 1) * 512], wslice(widx),
                                zr_in[:, cj * FD + hb + p * 512: cj * FD + hb + (p + 1) * 512],
                                start=(bi == 0),
                                stop=(bi == len(mlist) - 1),
                                skip_group_check=True,
                            )
                    if "dve" not in SKIP[0]:
                        # v = (psum - z*(k1/k2)) * (-k2)
                        vt = pool2.tile([128, HW], mybir.dt.float32, tag="v")
                        nc.vector._custom_dve(FISTA_V, out=vt[:], in0=ps[:],
                                              in1=z_t[:, cs], s0=k1 / k2, s1=-k2)
                        # m = min(v + s*y, y)
                        nc.vector._custom_dve(FISTA_M, out=m_out[:, cs], in0=vt[:],
                                              in1=y_t[:, cs], s0=s)
                        # z' = b1*relu(m) - b0*relu(m_prev)
                        nc.vector._custom_dve(FISTA_Z, out=z_t[:, cs], in0=m_out[:, cs],
                                              in1=m_in[:, cs], s0=i_b1, s1=i_b0)
                    if "gps" not in SKIP[0]:
                        # zr = f32r(z')
                        nc.gpsimd.tensor_copy(zr_out[:, cs], z_t[:, cs])

            def iteration2(m_in, m_out, zr_in, zr_out, i_b1, i_b0):
                for h in range(NHALF[0]):
                    iteration(m_in, m_out, zr_in, zr_out, h, i_b1, i_b0)

            def run_pass():
                nit = N_ITERS_RUN[0]
                if LOOP_MODE[0] == "unrolled":
                    for k in range(0, nit, 2):
                        iteration2(m_a, m_b, zr_a, zr_b, bet_t[:, k:k + 1],
                                  bet_t[:, N_ITERS + k:N_ITERS + k + 1])
                        iteration2(m_b, m_a, zr_b, zr_a, bet_t[:, k + 1:k + 2],
                                  bet_t[:, N_ITERS + k + 1:N_ITERS + k + 2])
                else:
                    nb = BODY[0]
                    assert nb % 2 == 0 and nit % nb == 0
                    def body(i):
                        for j in range(0, nb, 2):
                            iteration2(m_a, m_b, zr_a, zr_b,
                                      bet_t[:, bass.ds(i + j, 1)],
                                      bet_t[:, bass.ds(i + j + N_ITERS, 1)])
                            iteration2(m_b, m_a, zr_b, zr_a,
                                      bet_t[:, bass.ds(i + j + 1, 1)],
                                      bet_t[:, bass.ds(i + j + 1 + N_ITERS, 1)])
                    with tc.For_i(0, nit, nb) as i:
                        body(i)

            init_state()
            run_pass()
            for _extra in range(PASSES[0] - 1):
                # next pass: y <- relu(m_a) (x_100), reinit, run again
                nc.scalar.activation(y_t[:], m_a[:], mybir.ActivationFunctionType.Relu)
                init_state()
                run_pass()
            # final x = relu(m_a) -> z_t as staging
            if not DUMP_PS[0]:
                nc.scalar.activation(z_t[:], m_a[:], mybir.ActivationFunctionType.Relu)

            # store: transpose back on PE then contiguous DMA
            for i in range(NIMG_PER_CORE):
                for t_ in range(4):
                    psT = psp.tile([128, NCHUNK * GFD], f32, tag="ps")
                    for c in range(NCHUNK):
                        nc.tensor.transpose(
                            psT[:, 128 * c:128 * (c + 1)],
                            z_t[:, c * FD + i * N + 128 * t_: c * FD + i * N + 128 * (t_ + 1)],
                            ident)
                    S = m_b[:, ((i * 4 + t_) % 4) * N:(((i * 4 + t_) % 4) + 1) * N]
                    nc.vector.tensor_copy(S, psT[:, :N])
                    nc.sync.dma_start(OUT[i, 128 * t_:128 * (t_ + 1), :], S)

    nc.finalize()
    _cache[key] = nc
    return nc


def _build_fold(nc_args, ops, consts, key):
    """v3: identical math to legacy, restructured for engine efficiency.
    Per chunk per iteration:
      ps = A-blocks @ zr_in         (PE, f32r)
      vt = k1*z - k2*ps             (FISTA_V on DVE, fp32 z state: exact)
      x_out = relu(min(vt+s*y, y))  (FISTA_XR on DVE, fused clip)
      z = b1*x_out - b0*x_in        (FISTA_ZP on DVE, fp32)
      zr_out = f32r(z)              (ACT copy -- 2x faster than the legacy gpsimd
                                     copy and on an otherwise-idle engine)"""
    bacc, tile, mybir, bass = nc_args
    FISTA_V, FISTA_XR, FISTA_ZP = ops
    k1, k2, s = consts
    f32 = mybir.dt.float32
    f32r = mybir.dt.float32r

    nc = bacc.Bacc("TRN2", target_bir_lowering=False, debug=False)
    DATA = nc.dram_tensor("data", [NIMG_PER_CORE, N, N], f32, kind="ExternalInput")
    WTS = nc.dram_tensor("wts", [128, 6 * 128], f32, kind="ExternalInput")
    BET = nc.dram_tensor("betas", [128, 2 * N_ITERS], f32, kind="ExternalInput")
    OUT = nc.dram_tensor("out", [NIMG_PER_CORE, N, N], f32, kind="ExternalOutput")

    with tile.TileContext(nc) as tc:
        with (
            tc.tile_pool(name="state", bufs=1) as pool,
            tc.tile_pool(name="psum", bufs=2, space="PSUM") as psp,
            tc.tile_pool(name="scratch", bufs=1) as pool2,
        ):
            y_t = pool.tile([128, TOT], f32, tag="y")
            x_a = pool.tile([128, TOT], f32, tag="xa")
            x_b = pool.tile([128, TOT], f32, tag="xb")
            z_t = pool.tile([128, TOT], f32, tag="z")
            zr_a = pool.tile([128, TOT], f32r, tag="zra")
            zr_b = pool.tile([128, TOT], f32r, tag="zrb")
            w_t = pool.tile([128, 6 * 128], f32, tag="w")
            wr_t = pool.tile([128, 5 * 128], f32r, tag="wr")
            bet_t = pool.tile([128, 2 * N_ITERS], f32, tag="bet")

            nc.sync.dma_start(w_t[:], WTS[:])
            nc.sync.dma_start(bet_t[:], BET[:])
            nc.vector.tensor_copy(wr_t[:], w_t[:, :5 * 128])
            ident = w_t[:, 5 * 128:6 * 128]
            WBLK = {0: 0, 1: 1, 2: 1, 3: 2}

            def wslice(idx):
                return wr_t[:, idx * 128:(idx + 1) * 128]

            # load + transpose: y[p, c*FD + i*N + (128t+r')] = data[i, 128t+r', 128c+p]
            for i in range(NIMG_PER_CORE):
                for t_ in range(4):
                    sbase = ((i * 4 + t_) % 4) * N
                    nc.sync.dma_start(x_b[:, sbase:sbase + N],
                                      DATA[i, 128 * t_:128 * (t_ + 1), :])
                    psT = psp.tile([128, NCHUNK * GFD], f32, tag="ps")
                    for c in range(NCHUNK):
                        nc.tensor.transpose(psT[:, 128 * c:128 * (c + 1)],
                                            x_b[:, sbase + 128 * c:sbase + 128 * (c + 1)], ident)
                    dstv = y_t[:].rearrange("p (c n) -> p c n", c=NCHUNK)[
                        :, :, i * N + 128 * t_: i * N + 128 * (t_ + 1)]
                    nc.vector.tensor_copy(
                        dstv, psT[:, :N].rearrange("p (c n) -> p c n", c=NCHUNK))

            def init_state(src):
                # x0 = proj(y) = y (y >= 0); z0 = x0
                if src is not x_a:
                    nc.vector.tensor_copy(x_a[:], src[:])
                nc.vector.tensor_copy(z_t[:], src[:])
                nc.scalar.copy(zr_a[:], src[:])

            HW = FD // NHALF[0]

            def iteration(x_in, x_out, zr_in, zr_out, h, i_b1, i_b0):
                hb = h * HW
                for c in range(NCHUNK):
                    cs = slice(c * FD + hb, c * FD + hb + HW)
                    ps = psp.tile([128, HW], mybir.dt.float32, tag="ps")
                    mlist = [(WBLK[c], c)]
                    if "offdiag" not in SKIP[0]:
                        if c + 1 < NCHUNK:
                            mlist.append((3, c + 1))
                        if c - 1 >= 0:
                            mlist.append((4, c - 1))
                    if "mm" in SKIP[0]:
                        mlist = []
                        nc.scalar.activation(ps[:], y_t[:, cs], mybir.ActivationFunctionType.Copy)
                    for p in range(HW // 512):
                        for bi, (widx, cj) in enumerate(mlist):
                            nc.tensor.matmul(
                                ps[:, p * 512:(p + 1) * 512], wslice(widx),
                                zr_in[:, cj * FD + hb + p * 512: cj * FD + hb + (p + 1) * 512],
                                start=(bi == 0),
                                stop=(bi == len(mlist) - 1),
                                skip_group_check=True,
                            )
                    if "dve" not in SKIP[0]:
                        vt = pool2.tile([128, HW], mybir.dt.float32, tag="v")
                        # vt = k1*z - k2*ps
                        nc.vector._custom_dve(FISTA_V, out=vt[:], in0=ps[:],
                                              in1=z_t[:, cs], s0=k1 / k2, s1=-k2)
                        # x = relu(min(vt + s*y, y))
                        nc.vector._custom_dve(FISTA_XR, out=x_out[:, cs], in0=vt[:],
                                              in1=y_t[:, cs], s0=s)
                        # z' = b1*x_new - b0*x_old
                        nc.vector._custom_dve(FISTA_ZP, out=z_t[:, cs], in0=x_out[:, cs],
                                              in1=x_in[:, cs], s0=i_b1, s1=i_b0)
                    if "zr" not in SKIP[0]:
                        # zr = f32r(z) on the idle ACT engine
                        nc.scalar.copy(zr_out[:, cs], z_t[:, cs])

            def iteration2(x_in, x_out, zr_in, zr_out, i_b1, i_b0):
                for h in range(NHALF[0]):
                    iteration(x_in, x_out, zr_in, zr_out, h, i_b1, i_b0)

            def run_pass():
                nit = N_ITERS_RUN[0]
                if LOOP_MODE[0] == "unrolled":
                    for k in range(0, nit, 2):
                        iteration2(x_a, x_b, zr_a, zr_b, bet_t[:, k:k + 1],
                                   bet_t[:, N_ITERS + k:N_ITERS + k + 1])
                        iteration2(x_b, x_a, zr_b, zr_a, bet_t[:, k + 1:k + 2],
                                   bet_t[:, N_ITERS + k + 1:N_ITERS + k + 2])
                else:
                    nb = BODY[0]
                    assert nb % 2 == 0 and nit % nb == 0
                    def body(i):
                        for j in range(0, nb, 2):
                            iteration2(x_a, x_b, zr_a, zr_b,
                                       bet_t[:, bass.ds(i + j, 1)],
                                       bet_t[:, bass.ds(i + j + N_ITERS, 1)])
                            iteration2(x_b, x_a, zr_b, zr_a,
                                       bet_t[:, bass.ds(i + j + 1, 1)],
                                       bet_t[:, bass.ds(i + j + 1 + N_ITERS, 1)])
                    with tc.For_i(0, nit, nb) as i:
                        body(i)

            init_state(y_t)
            run_pass()
            for _extra in range(PASSES[0] - 1):
                # next pass: y <- x_100 (already >= 0 and x0 of the next pass)
                nc.vector.tensor_copy(y_t[:], x_a[:])
                init_state(x_a)
                run_pass()

            # store: transpose back on PE then contiguous DMA (x_a is the result)
            for i in range(NIMG_PER_CORE):
                for t_ in range(4):
                    psT = psp.tile([128, NCHUNK * GFD], f32, tag="ps")
                    for c in range(NCHUNK):
                        nc.tensor.transpose(
                            psT[:, 128 * c:128 * (c + 1)],
                            x_a[:, c * FD + i * N + 128 * t_: c * FD + i * N + 128 * (t_ + 1)],
                            ident)
                    S = x_b[:, ((i * 4 + t_) % 4) * N:(((i * 4 + t_) % 4) + 1) * N]
                    nc.vector.tensor_copy(S, psT[:, :N])
                    nc.sync.dma_start(OUT[i, 128 * t_:128 * (t_ + 1), :], S)

    nc.finalize()
    _cache[key] = nc
    return nc


def kernel(data: np.ndarray) -> np.ndarray:
    from concourse import bass_utils

    data = np.ascontiguousarray(data, np.float32)
    B = data.shape[0]
    nc = _build()
    _, _, _, _, _, wts_np, betas_np = _host_consts()
    in_maps = []
    for c in range(NCORES):
        in_maps.append({
            "data": np.ascontiguousarray(data[c * NIMG_PER_CORE:(c + 1) * NIMG_PER_CORE]),
            "wts": wts_np,
            "betas": betas_np,
        })
    res = bass_utils.run_bass_kernel_spmd(nc, in_maps, core_ids=list(range(NCORES)))
    out = np.concatenate([res.results[c]["out"] for c in range(NCORES)], axis=0)
    return out.reshape(B, N, N, 1)


if __name__ == "__main__":
    rng = np.random.default_rng(0)
    d = rng.random((32, N, N), dtype=np.float32)
    o = kernel(d)
    print("kernel ran, out shape", o.shape, "mean", o.mean())

